# revision 12
# baseline (speedup 1.0000x reference)
"""GNN message-passing encoder (nn_Encoder_52252572123266) on 8 TRN2 NeuronCores.

Strategy: receiver-range edge sharding (core k owns nodes [2500k, 2500(k+1))
and every edge whose receiver lies there), so segment_sum needs no cross-core
reduction; only updated node features x are AllGathered between rounds.

v2 architecture (vs. the dma_gather/PE-transpose baseline):
  - x lives FEATURE-MAJOR in SBUF ([128 feats, 20480 global slots] fp32).
    Endpoint features are fetched per edge with the GPSIMD `ap_gather`
    SBUF free-dim gather (out = in[:, idxs]), which lands feature-major
    directly: no DMA descriptors, no PE transposes, no PSUM round trips.
  - Edge features stream through DRAM in bf16; matmuls are fp32r except
    the ec k-tile (bf16 weights) and the scatter (bf16 one-hot).
  - segment_sum: edges sorted by receiver into windows of 128 node slots,
    each window padded to a cross-core-uniform multiple of 128 edges so a
    single SPMD program fits every core (no data-dependent control flow);
    scatter = PE transpose of e_new + one-hot matmul accumulating into a
    [128,128] PSUM window tile.
  - The inter-round AllGather is split into 5 slices, each fired as soon
    as its node-MLP chunk completes (node chunks are emitted inline as
    their agg windows close), so the collective overlaps the edge loop.
"""

import math
import os
from contextlib import ExitStack

import numpy as np

import concourse.bass as bass  # noqa: F401  (import keeps bass registered)
import concourse.tile as tile
from concourse import bacc, mybir
from concourse.bass_utils import run_bass_kernel_spmd

P = 128
N_CORES = 8
N = 20000
E = 320000
D = 128
H = 256
NUM_FINE = 2

NODES_PER_CORE = N // N_CORES            # 2500
WIN = 128                                 # node slots per scatter window
N_WIN = math.ceil(NODES_PER_CORE / WIN)   # 20
NODE_SLOTS = N_WIN * WIN                  # 2560 padded node slots per core
TOT_SLOTS = N_CORES * NODE_SLOTS          # 20480
GB = 1024                                 # idxs per ap_gather call
CHUNK = 512                               # edges per MLP chunk
NSL = NODE_SLOTS // CHUNK                 # 5 node chunks / allgather slices

F32 = mybir.dt.float32
F32R = mybir.dt.float32r
BF16 = mybir.dt.bfloat16
I16 = mybir.dt.int16
RELU = mybir.ActivationFunctionType.Relu


# ----------------------------------------------------------------------------
# Host-side preparation
# ----------------------------------------------------------------------------

def _pad_slot(ids):
    """global node id -> padded-global slot id (core-major, NODE_SLOTS per core)."""
    return NODE_SLOTS * (ids // NODES_PER_CORE) + ids % NODES_PER_CORE


def _wrap_idx16(slots, ep):
    """ap_gather index layout: per GB-block, idx i at [i%16, blk*(GB/16)+i//16],
    replicated across the 8 groups of 16 partitions -> [128, ep/16] int16."""
    flat = np.zeros(ep, dtype=np.int16)
    flat[: len(slots)] = slots.astype(np.int16)
    out = np.zeros((P, ep // 16), dtype=np.int16)
    for b in range(ep // GB):
        blk = flat[b * GB : (b + 1) * GB].reshape(GB // 16, 16).T  # [16, GB/16]
        out[:, b * (GB // 16) : (b + 1) * (GB // 16)] = np.tile(blk, (8, 1))
    return np.ascontiguousarray(out)


def _window_layout(receivers):
    """Cross-core-uniform per-window padded edge counts m_w (multiples of 128),
    with the last window absorbing GB alignment."""
    rslot = _pad_slot(receivers)
    wloc = rslot % NODE_SLOTS // WIN
    core = rslot // NODE_SLOTS
    m = np.zeros(N_WIN, dtype=np.int64)
    for w in range(N_WIN):
        cnt = np.bincount(core[wloc == w], minlength=N_CORES)
        m[w] = int(math.ceil((cnt.max() + 1) / P) * P)
    ep = int(m.sum())
    m[N_WIN - 1] += (-ep) % GB
    return m, ep + (-ep) % GB


def prepare_core(k, nodes, edges, senders, receivers, m_w, ep):
    lo = k * NODES_PER_CORE
    hi = lo + NODES_PER_CORE
    eids = np.nonzero((receivers >= lo) & (receivers < hi))[0]
    rloc = receivers[eids] - lo
    order = np.argsort(rloc, kind="stable")
    eids = eids[order]
    rloc = rloc[order]
    w = rloc // WIN

    send_slots = np.zeros(ep, dtype=np.int64)        # pad -> slot 0
    recv_slots = np.zeros(ep, dtype=np.int64)        # pad -> slot 0
    rrel = np.full(ep, -1.0, dtype=np.float32)       # pad -> -1 (sel row = 0)
    perm = np.full(ep, -1, dtype=np.int64)           # stream pos -> edge id
    base = 0
    for wi in range(N_WIN):
        sel = w == wi
        cnt = int(sel.sum())
        assert cnt <= m_w[wi], (k, wi, cnt, m_w[wi])
        perm[base : base + cnt] = eids[sel]
        send_slots[base : base + cnt] = _pad_slot(senders[eids[sel]])
        recv_slots[base : base + cnt] = _pad_slot(receivers[eids[sel]])
        rrel[base : base + cnt] = (rloc[sel] - wi * WIN).astype(np.float32)
        base += int(m_w[wi])
    assert base == ep

    edges_T = np.zeros((D, ep), dtype=np.float32)
    real = perm >= 0
    edges_T[:, real] = edges[perm[real]].T

    nodes_T = np.zeros((D, NODE_SLOTS), dtype=np.float32)
    nodes_T[:, :NODES_PER_CORE] = nodes[lo:hi].T

    return dict(
        edges_T=edges_T,
        nodes_T=nodes_T,
        send_idx=_wrap_idx16(send_slots, ep),
        recv_idx=_wrap_idx16(recv_slots, ep),
        rrel=np.ascontiguousarray(rrel.reshape(ep // P, P).T),
    )


def build_weight_blob(ws):
    """Concatenate weight k-tile blocks + ones column -> (128, WCOLS) fp32."""
    cols = []
    offs = {}
    c = 0
    for name, wmat in ws.items():
        K, M = wmat.shape
        for kt in range(K // P):
            cols.append(np.asarray(wmat[kt * P : (kt + 1) * P, :], dtype=np.float32))
        offs[name] = (c, M)
        c += (K // P) * M
    offs["ones"] = (c, 1)
    cols.append(np.ones((P, 1), dtype=np.float32))
    c += 1
    return np.concatenate(cols, axis=1), offs


def build_aux_blob(bs):
    """Biases (one [128,1] col per m-tile) + iota row -> (128, cols) fp32."""
    cols = []
    offs = {}
    c = 0
    for name, b in bs.items():
        b = np.asarray(b, dtype=np.float32)
        nmt = len(b) // P
        cols.append(b.reshape(nmt, P).T)
        offs[name] = c
        c += nmt
    offs["iota"] = c
    cols.append(np.tile(np.arange(WIN, dtype=np.float32)[None, :], (P, 1)))
    c += WIN
    return np.concatenate(cols, axis=1), offs


# ----------------------------------------------------------------------------
# Bass program
# ----------------------------------------------------------------------------

def build_program(ep, m_w, wcols, acols, woffs, aoffs):
    n_chunk = ep // CHUNK
    n_group = ep // P
    n_blk = ep // GB
    knocc = os.environ.get("KNOCC") == "1"

    # group -> window map (identical on every core by construction)
    group_win = np.repeat(np.arange(N_WIN), np.asarray(m_w) // P)
    assert len(group_win) == n_group
    win_g0 = {}
    win_g1 = {}
    for g, w in enumerate(group_win):
        win_g0.setdefault(int(w), g)
        win_g1[int(w)] = g

    nc = bacc.Bacc(None, target_bir_lowering=False, debug=False)

    edges_T = nc.dram_tensor("edges_T", [P, ep], F32, kind="ExternalInput")
    nodes_T = nc.dram_tensor("nodes_T", [P, NODE_SLOTS], F32, kind="ExternalInput")
    send_idx = nc.dram_tensor("send_idx", [P, ep // 16], I16, kind="ExternalInput")
    recv_idx = nc.dram_tensor("recv_idx", [P, ep // 16], I16, kind="ExternalInput")
    rrel_in = nc.dram_tensor("rrel", [P, n_group], F32, kind="ExternalInput")
    wblob = nc.dram_tensor("wblob", [P, wcols], F32, kind="ExternalInput")
    ablob = nc.dram_tensor("ablob", [P, acols], F32, kind="ExternalInput")
    castb = nc.dram_tensor("castb", [P, H + P], F32, kind="ExternalInput")
    out_pooled = nc.dram_tensor(
        "out_pooled", [NUM_FINE, NODE_SLOTS], F32, kind="ExternalOutput"
    )

    with tile.TileContext(nc) as tc, ExitStack() as ctx:
        sb1 = ctx.enter_context(tc.tile_pool(name="sb1", bufs=1))
        dram = ctx.enter_context(tc.tile_pool(name="dram", bufs=1, space="DRAM"))
        pedg = ctx.enter_context(tc.tile_pool(name="pedg", bufs=3))
        pec = ctx.enter_context(tc.tile_pool(name="pec", bufs=3))
        pgx = ctx.enter_context(tc.tile_pool(name="pgx", bufs=2))
        pgr = ctx.enter_context(tc.tile_pool(name="pgr", bufs=3))
        pml = ctx.enter_context(tc.tile_pool(name="pml", bufs=4))
        pe1 = ctx.enter_context(tc.tile_pool(name="pe1", bufs=3))
        pet = ctx.enter_context(tc.tile_pool(name="pet", bufs=3))
        psel = ctx.enter_context(tc.tile_pool(name="psel", bufs=6))
        pagg = ctx.enter_context(tc.tile_pool(name="pagg", bufs=NSL))
        ppool = ctx.enter_context(tc.tile_pool(name="ppool", bufs=2))
        ph = ctx.enter_context(tc.tile_pool(name="ph", bufs=2, space="PSUM"))
        pe2 = ctx.enter_context(tc.tile_pool(name="pe2", bufs=2, space="PSUM"))
        ptr = ctx.enter_context(tc.tile_pool(name="ptr", bufs=2, space="PSUM"))
        pag_ps = ctx.enter_context(tc.tile_pool(name="pag", bufs=2, space="PSUM"))

        # ---------------- resident tiles ----------------
        wsb = sb1.tile([P, wcols], F32R)
        nc.gpsimd.dma_start(wsb[:], wblob[:].bitcast(F32R))
        asb = sb1.tile([P, acols], F32)
        nc.gpsimd.dma_start(asb[:], ablob[:])
        cstg = sb1.tile([P, H + P], F32)
        nc.gpsimd.dma_start(cstg[:], castb[:])
        cbf = sb1.tile([P, H + P], BF16)
        nc.vector.tensor_copy(cbf[:], cstg[:])
        w1e_bf = cbf[:, 0:H]          # Wed1 k-tile 2 (edge-attr rows), bf16
        ident_bf = cbf[:, H : H + P]  # bf16 identity for transposes
        sidx = sb1.tile([P, ep // 16], I16)
        nc.gpsimd.dma_start(sidx[:], send_idx[:])
        ridx = sb1.tile([P, ep // 16], I16)
        nc.gpsimd.dma_start(ridx[:], recv_idx[:])
        rrel = sb1.tile([P, n_group], F32)
        nc.gpsimd.dma_start(rrel[:], rrel_in[:])

        xfull = sb1.tile([P, TOT_SLOTS], F32, name="xfull", tag="xfull")
        x_ping = [
            sb1.tile([P, NODE_SLOTS], F32R, name=f"xloc{i}", tag=f"xloc{i}")
            for i in range(2)
        ]

        def w_ap(name, kt):
            c, m = woffs[name]
            return wsb[:, c + kt * m : c + (kt + 1) * m]

        ones_col = w_ap("ones", 0)

        def b_ap(name, mt):
            c = aoffs[name]
            return asb[:, c + mt : c + mt + 1]

        iota_f = asb[:, aoffs["iota"] : aoffs["iota"] + WIN]

        # DRAM intermediates
        e_a = dram.tile([P, ep], BF16, tag="ea")
        e_b = dram.tile([P, ep], BF16, tag="eb")
        x_pad0 = dram.tile([P, NODE_SLOTS], F32, tag="xp0")
        x_g0 = dram.tile(
            [N_CORES * P, NODE_SLOTS], F32, tag="xg0",
            addr_space=("Local" if knocc else "Shared"),
        )
        x_pad1 = [dram.tile([P, CHUNK], F32, name=f"xp1_{c}", tag=f"xp1_{c}")
                  for c in range(NSL)]
        x_g1 = [
            dram.tile([N_CORES * P, CHUNK], F32, name=f"xg1_{c}", tag=f"xg1_{c}",
                      addr_space=("Local" if knocc else "Shared"))
            for c in range(NSL)
        ]

        # PE warmup: absorb fresh semaphores.
        wu = ptr.tile([P, CHUNK], BF16, tag="tr")
        nc.tensor.matmul(wu[:, 0:P], ident_bf, ident_bf, is_transpose=True)

        def relu_act(dst_ap, src_ap, bias):
            nc.scalar.activation(dst_ap, src_ap, RELU, bias=bias)

        def relu_dve(dst_ap, src_ap, bias):
            nc.vector.tensor_scalar(
                out=dst_ap, in0=src_ap, scalar1=bias, scalar2=0.0,
                op0=mybir.AluOpType.add, op1=mybir.AluOpType.max,
            )

        def mlp_l1(rhs_list, w1, b1):
            """Layer 1 on one 512-col chunk -> [ht0, ht1] f32r SBUF tiles.
            rhs_list entries: (lhs_ap_fn(mt), rhs_ap)."""
            hts = []
            for mt in range(2):
                hp = ph.tile([P, CHUNK], F32, tag="h")
                for i, (lhs_fn, rhs) in enumerate(rhs_list):
                    nc.tensor.matmul(
                        hp[:], lhs_fn(mt), rhs,
                        start=(i == 0), stop=(i == len(rhs_list) - 1),
                    )
                ht = pml.tile([P, CHUNK], F32R, tag="hsb")
                if mt == 0:
                    relu_act(ht[:], hp[:], b_ap(b1, mt))
                else:
                    relu_dve(ht[:], hp[:], b_ap(b1, mt))
                hts.append(ht)
            return hts

        def mlp_l2(hts, w2, b2, dst_ap, engine="act"):
            ep2 = pe2.tile([P, CHUNK], F32, tag="eps")
            for kt in range(2):
                nc.tensor.matmul(ep2[:], w_ap(w2, kt), hts[kt][:],
                                 start=(kt == 0), stop=(kt == 1))
            if engine == "act":
                relu_act(dst_ap, ep2[:], b_ap(b2, 0))
            else:
                relu_dve(dst_ap, ep2[:], b_ap(b2, 0))

        def wslice(name, kt):
            return lambda mt, name=name, kt=kt: w_ap(name, kt)[:, mt * P : (mt + 1) * P]

        def gather(out_tile, idx_tile, blk):
            nc.gpsimd.ap_gather(
                out_ap=out_tile[:].rearrange("p (n d) -> p n d", d=1),
                in_ap=xfull[:].rearrange("p (n d) -> p n d", d=1),
                idxs_ap=idx_tile[:, blk * (GB // 16) : (blk + 1) * (GB // 16)],
                channels=P,
                num_elems=TOT_SLOTS,
                d=1,
                num_idxs=GB,
            )

        # ---------------- node embed ----------------
        x_own = x_ping[0]
        for cn in range(NSL):
            sl = slice(cn * CHUNK, (cn + 1) * CHUNK)
            nt = pedg.tile([P, CHUNK], F32R, tag="ein")
            nc.sync.dma_start(nt[:], nodes_T[:, sl].bitcast(F32R))
            hts = mlp_l1([(wslice("Wn1", 0), nt[:])], "Wn1", "bn1")
            mlp_l2(hts, "Wn2", "bn2", x_own[:, sl])
        nc.sync.dma_start(x_pad0[:].bitcast(F32R), x_own[:])
        if knocc:
            nc.gpsimd.dma_start(x_g0[0:P, :], x_pad0[:])
        else:
            nc.gpsimd.collective_compute(
                "AllGather", mybir.AluOpType.bypass,
                ins=[x_pad0.opt()], outs=[x_g0.opt()],
                replica_groups=[list(range(N_CORES))],
            )
        for b in range(N_CORES):
            nc.sync.dma_start(
                xfull[:, b * NODE_SLOTS : (b + 1) * NODE_SLOTS],
                x_g0[b * P : (b + 1) * P, :],
            )

        # ---------------- edge embed ----------------
        for mc in range(n_chunk):
            sl = slice(mc * CHUNK, (mc + 1) * CHUNK)
            et = pedg.tile([P, CHUNK], F32R, tag="ein")
            nc.sync.dma_start(et[:], edges_T[:, sl].bitcast(F32R))
            hts = mlp_l1([(wslice("We1", 0), et[:])], "We1", "be1")
            e0 = pe1.tile([P, CHUNK], BF16, tag="e1")
            mlp_l2(hts, "We2", "be2", e0[:])
            nc.sync.dma_start(e_a[:, sl], e0[:])

        # ---------------- fine iterations ----------------
        _tick = [0]

        def node_chunk(t, cn, x_own, x_next, agg_sb):
            """Node MLP for 512-slot chunk cn of iteration t, emitted inline."""
            sl = slice(cn * CHUNK, (cn + 1) * CHUNK)
            hts = mlp_l1(
                [(wslice("Wnd1", 0), x_own[:, sl]),
                 (wslice("Wnd1", 1), agg_sb[:])],
                "Wnd1", "bnd1",
            )
            mlp_l2(hts, "Wnd2", "bnd2", x_next[:, sl])
            pp = pe2.tile([1, CHUNK], F32, tag="eps")
            nc.tensor.matmul(pp[:], ones_col, x_next[:, sl],
                             start=True, stop=True)
            po = ppool.tile([1, CHUNK], F32, tag="po")
            nc.vector.tensor_copy(po[:], pp[:])
            nc.scalar.dma_start(out_pooled[t : t + 1, sl], po[:])
            if t < NUM_FINE - 1:
                nc.scalar.dma_start(x_pad1[cn][:].bitcast(F32R), x_next[:, sl])
                if knocc:
                    nc.gpsimd.dma_start(x_g1[cn][0:P, :], x_pad1[cn][:])
                else:
                    nc.gpsimd.collective_compute(
                        "AllGather", mybir.AluOpType.bypass,
                        ins=[x_pad1[cn].opt()], outs=[x_g1[cn].opt()],
                        replica_groups=[list(range(N_CORES))],
                    )

        for t in range(NUM_FINE):
            e_in = e_a if t == 0 else e_b
            x_own = x_ping[t % 2]
            x_next = x_ping[(t + 1) % 2]
            ag_ps = {}
            agg_sb = {}
            xs_g = xr_g = None

            for mc in range(n_chunk):
                blk, sub = divmod(mc, GB // CHUNK)
                if sub == 0:
                    xs_f = pgx.tile([P, GB], F32, tag="xsf")
                    xr_f = pgx.tile([P, GB], F32, tag="xrf")
                    gather(xs_f, sidx, blk)
                    gather(xr_f, ridx, blk)
                    # SBUF->SBUF DMA re-tags the gather output as fp32r for
                    # the PE (bit copy; DMACopy is a legal fp32r producer).
                    xs_g = pgr.tile([P, GB], F32R, tag="xs")
                    xr_g = pgr.tile([P, GB], F32R, tag="xr")
                    nc.gpsimd.dma_start(xs_g[:], xs_f[:].bitcast(F32R))
                    nc.gpsimd.dma_start(xr_g[:], xr_f[:].bitcast(F32R))
                sl = slice(mc * CHUNK, (mc + 1) * CHUNK)
                ec = pec.tile([P, CHUNK], BF16, tag="ec")
                nc.sync.dma_start(ec[:], e_in[:, sl])
                xs = xs_g[:, sub * CHUNK : (sub + 1) * CHUNK]
                xr = xr_g[:, sub * CHUNK : (sub + 1) * CHUNK]
                hts = mlp_l1(
                    [(wslice("Wed1", 0), xs),
                     (wslice("Wed1", 1), xr),
                     (lambda mt: w1e_bf[:, mt * P : (mt + 1) * P], ec[:])],
                    "Wed1", "bed1",
                )
                e1 = pe1.tile([P, CHUNK], BF16, tag="e1")
                mlp_l2(hts, "Wed2", "bed2", e1[:])
                if t < NUM_FINE - 1:
                    nc.sync.dma_start(e_b[:, sl], e1[:])
                # transpose e1 -> edge-major (4x [128,128] into one psum tile)
                etp = ptr.tile([P, CHUNK], BF16, tag="tr")
                for j in range(4):
                    nc.tensor.matmul(
                        etp[:, j * P : (j + 1) * P], e1[:, j * P : (j + 1) * P],
                        ident_bf, is_transpose=True, skip_group_check=True,
                    )
                eT = pet.tile([P, CHUNK], BF16, tag="eT")
                _tick[0] ^= 1
                if _tick[0]:
                    nc.vector.tensor_copy(eT[:], etp[:])
                else:
                    nc.scalar.copy(eT[:], etp[:])
                # scatter: per 128-edge group, one-hot matmul into window psum
                for j in range(4):
                    g = mc * 4 + j
                    w = int(group_win[g])
                    if win_g0[w] == g:
                        ag_ps[w] = pag_ps.tile([P, WIN], F32, name=f"agps{w % 2}", tag="agps")
                    selt = psel.tile([P, WIN], BF16, tag="sel")
                    nc.vector.tensor_tensor(
                        out=selt[:],
                        in0=rrel[:, g : g + 1].to_broadcast([P, WIN]),
                        in1=iota_f,
                        op=mybir.AluOpType.is_equal,
                    )
                    nc.tensor.matmul(
                        ag_ps[w][:], eT[:, j * P : (j + 1) * P], selt[:],
                        start=(win_g0[w] == g), stop=(win_g1[w] == g),
                        skip_group_check=True,
                    )
                    if win_g1[w] == g:
                        cn = w // 4
                        if w % 4 == 0:
                            agg_sb[cn] = pagg.tile([P, CHUNK], F32R, name=f"agg{cn}", tag="agg")
                        nc.vector.tensor_copy(
                            agg_sb[cn][:, (w % 4) * P : (w % 4 + 1) * P],
                            ag_ps.pop(w)[:],
                        )
                        if w % 4 == 3:
                            node_chunk(t, cn, x_own, x_next, agg_sb.pop(cn))

            if t < NUM_FINE - 1:
                for cn in range(NSL):
                    for b in range(N_CORES):
                        nc.sync.dma_start(
                            xfull[:, b * NODE_SLOTS + cn * CHUNK :
                                  b * NODE_SLOTS + (cn + 1) * CHUNK],
                            x_g1[cn][b * P : (b + 1) * P, :],
                        )

    nc.compile()
    return nc


# ----------------------------------------------------------------------------
# Entry point
# ----------------------------------------------------------------------------

def _prepare(inputs):
    nodes = np.asarray(inputs["nodes"], dtype=np.float32)
    edges = np.asarray(inputs["edges"], dtype=np.float32)
    senders = np.asarray(inputs["senders"]).astype(np.int64)
    receivers = np.asarray(inputs["receivers"]).astype(np.int64)

    ws = {k: np.asarray(inputs[k]) for k in
          ["Wn1", "Wn2", "We1", "We2", "Wed1", "Wed2", "Wnd1", "Wnd2"]}
    bs = {k: np.asarray(inputs[k]) for k in
          ["bn1", "bn2", "be1", "be2", "bed1", "bed2", "bnd1", "bnd2"]}

    m_w, ep = _window_layout(receivers)

    wblob, woffs = build_weight_blob(ws)
    ablob, aoffs = build_aux_blob(bs)
    castb = np.concatenate(
        [np.asarray(ws["Wed1"][2 * P : 3 * P, :], dtype=np.float32),
         np.eye(P, dtype=np.float32)],
        axis=1,
    )

    in_maps = []
    for k in range(N_CORES):
        m = prepare_core(k, nodes, edges, senders, receivers, m_w, ep)
        m["wblob"] = wblob
        m["ablob"] = ablob
        m["castb"] = castb
        in_maps.append(m)

    nc = build_program(ep, m_w, wblob.shape[1], ablob.shape[1], woffs, aoffs)
    return nc, in_maps


def _assemble(results):
    out = np.zeros(NUM_FINE * N, dtype=np.float32)
    for k in range(N_CORES):
        pooled = results[k]["out_pooled"]
        for t in range(NUM_FINE):
            out[t * N + k * NODES_PER_CORE : t * N + (k + 1) * NODES_PER_CORE] = (
                pooled[t, :NODES_PER_CORE]
            )
    return out


def _run(inputs, trace=False):
    nc, in_maps = _prepare(inputs)
    res = run_bass_kernel_spmd(
        nc, in_maps, core_ids=list(range(N_CORES)), trace=trace
    )
    return _assemble(res.results), res


def kernel(**inputs):
    out, _ = _run(inputs, trace=False)
    return out
